# revision 1
# baseline (speedup 1.0000x reference)
"""
Trainium2 Bass kernel for nn_ARqGPS (autoregressive qGPS log-amplitude).

Math (validated vs reference):
  eps_sel[b,m,t] = epsilon[x[b,t], m, t]
  H[b,m,t]  = prod_{j<=t} eps_sel[b,m,j]        (log-space cumsum on device)
  r_picked[b,t] = sum_m H[b,m,t]
  r_sum[b,t]    = sum_m (eps0+eps1)[m,t] * H[b,m,t-1]   (H[.,.,-1] = 1)
  r_other = r_sum - r_picked
  term[b,t] = unmasked_other * (rp - mx - 0.5*log1p(exp(2*(mn-mx))))
  out[b] = sum_t term[b,t]

Device layout: t on partitions (2 chunks of 128), free = (b-major, m-minor).
  T1 = x*lr (DVE), S = Tri @ (T1 + l0_bcast) via PE psum accumulation
  H = exp(S) (ACT), S2 = S + lnw_shift_bcast (DVE), WH = exp(S2) (ACT)
  r_picked/r_sum_next = segmented reduce over m (DVE)
  shift/mask/logsumexp tail: small matmuls + DVE/ACT ops.

Sharding: data-parallel over batch, 128 rows per core, 8 cores.
"""
import os
import sys

import numpy as np

for _p in ("/opt/trn_rl_repo", os.path.expanduser("~/.axon_site/_ro/trn_rl_repo")):
    if os.path.isdir(_p) and _p not in sys.path:
        sys.path.insert(0, _p)
        break

import concourse.bass as bass
import concourse.bacc as bacc
import concourse.mybir as mybir
from concourse.tile import TileContext

B, L, M = 1024, 256, 128
NCORES = 8
BLOC = B // NCORES          # 128 batch rows per core
HALF = L // 2
NB = 4                      # batch rows per free-block
FB = NB * M                 # 512 free elements per matmul/psum tile
NBLK = BLOC // NB           # 32 blocks
NCHUNK = 2                  # t-chunks of 128 partitions

F32 = mybir.dt.float32
F32R = mybir.dt.float32r
AF = mybir.ActivationFunctionType
ALU = mybir.AluOpType

USE_F32R = True             # f32r: 1 cyc/row matmul vs fp32 4 cyc/row


def _r(ap):
    return ap.bitcast(F32R) if USE_F32R else ap


def build_nc():
    nc = bacc.Bacc("TRN2", target_bir_lowering=False)
    # all fp32 constants packed into one tensor (single DMA -> single wait sem),
    # f32r matmul operands packed into a second one
    meg = nc.dram_tensor("meg", (128, 1282), F32, kind="ExternalInput")
    megr = nc.dram_tensor("megr", (128, 512), F32R, kind="ExternalInput")
    one0 = nc.dram_tensor("one0", (1, BLOC), F32, kind="ExternalInput")
    cst0 = nc.dram_tensor("cst0", (1, BLOC), F32, kind="ExternalInput")
    y = nc.dram_tensor("y", (1, BLOC), F32, kind="ExternalOutput")

    with TileContext(nc) as tc:
        with (
            tc.tile_pool(name="const", bufs=1) as cpool,
            tc.tile_pool(name="t1p", bufs=3) as t1pool,
            tc.tile_pool(name="hp", bufs=2) as hpool,
            tc.tile_pool(name="whp", bufs=2) as whpool,
            tc.tile_pool(name="small", bufs=1) as spool,
            tc.tile_pool(name="ps", bufs=3, space="PSUM") as pspool,
            tc.tile_pool(name="psm", bufs=2, space="PSUM") as psmisc,
        ):
            # ------- constants into SBUF (2 packed DMAs) -------
            MEG = cpool.tile([128, 1282], F32, tag="MEG")
            MEGR = cpool.tile([128, 512], F32R, tag="MEGR")
            nc.sync.dma_start(MEG[:], meg[:])
            nc.sync.dma_start(MEGR[:], megr[:])
            X = MEG[:, 0:256].rearrange("p (c b) -> p c b", c=NCHUNK)
            LR = MEG[:, 256:512].rearrange("p (c m) -> p c m", c=NCHUNK)
            LW = MEG[:, 512:768].rearrange("p (c m) -> p c m", c=NCHUNK)
            ONESM = MEG[:, 768:896]
            STRI = MEG[:, 896:1024]
            SHM = MEG[:, 1024:1152]
            SH2 = MEG[:, 1152:1280]
            TV = MEG[:, 1280:1282]
            L0 = MEGR[:, 0:256].rearrange("p (c m) -> p c m", c=NCHUNK)
            TRI = MEGR[:, 256:384]
            ONESR = MEGR[:, 384:512]
            ONE0 = cpool.tile([1, BLOC], F32, tag="ONE0")
            CST0 = cpool.tile([1, BLOC], F32, tag="CST0")
            nc.sync.dma_start(ONE0[:], one0[:])
            nc.sync.dma_start(CST0[:], cst0[:])

            # persistent accumulators for the reduce outputs
            RP = spool.tile([128, NCHUNK, BLOC], F32, tag="RP")
            RSN = spool.tile([128, NCHUNK, BLOC], F32, tag="RSN")

            # ------- main blocked pipeline -------
            # wide blocks: 8 batch rows per SBUF-side instruction, matmul/exp
            # still work in 512-free (one PSUM bank) halves
            WB = 2 * NB
            for wb in range(BLOC // WB):
                bsl = slice(wb * WB, (wb + 1) * WB)
                t1eng = nc.gpsimd if wb % 3 == 0 else nc.vector
                t1 = []
                for c in range(NCHUNK):
                    t = t1pool.tile([128, WB, M], F32R, tag=f"T1_{c}")
                    xbc = X[:, c, bsl].unsqueeze(2).broadcast_to([128, WB, M])
                    lrbc = LR[:, c, :].unsqueeze(1).broadcast_to([128, WB, M])
                    t1eng.tensor_tensor(t[:], xbc, lrbc, ALU.mult)
                    t1.append(t)
                l0bc = [
                    L0[:, c, :].unsqueeze(1).broadcast_to([128, NB, M])
                    for c in range(NCHUNK)
                ]
                for c in range(NCHUNK):
                    ht = hpool.tile([128, WB, M], F32, tag=f"H_{c}")
                    for half in range(2):
                        hsl = slice(half * NB, (half + 1) * NB)
                        sp_ = pspool.tile([128, FB], F32, tag=f"S_{c}")
                        spv = sp_[:].rearrange("p (a b) -> p a b", b=M)
                        if c == 0:
                            nc.tensor.matmul(spv, TRI, t1[0][:, hsl, :],
                                             start=True, stop=False)
                            nc.tensor.matmul(spv, TRI, l0bc[0],
                                             start=False, stop=True)
                        else:
                            nc.tensor.matmul(spv, TRI, t1[1][:, hsl, :],
                                             start=True, stop=False)
                            nc.tensor.matmul(spv, TRI, l0bc[1],
                                             start=False, stop=False)
                            nc.tensor.matmul(spv, ONESR, t1[0][:, hsl, :],
                                             start=False, stop=False)
                            nc.tensor.matmul(spv, ONESR, l0bc[0],
                                             start=False, stop=True)
                        nc.scalar.activation(ht[:, hsl, :], spv, AF.Exp)
                    wh = whpool.tile([128, WB, M], F32, tag=f"WH_{c}")
                    wbc = LW[:, c, :].unsqueeze(1).broadcast_to([128, WB, M])
                    nc.gpsimd.tensor_tensor(wh[:], ht[:], wbc, ALU.mult)
                    nc.vector.tensor_reduce(RP[:, c, bsl], ht[:],
                                            mybir.AxisListType.X, ALU.add)
                    nc.vector.tensor_reduce(RSN[:, c, bsl], wh[:],
                                            mybir.AxisListType.X, ALU.add)

            # ------- tail -------
            # exclusive spin-up counts c1[t,b] via strict-lower-tri matmuls
            C1p = psmisc.tile([128, NCHUNK, BLOC], F32, tag="misc")
            nc.tensor.matmul(C1p[:, 0, :], STRI, X[:, 0, :],
                             start=True, stop=True)
            nc.tensor.matmul(C1p[:, 1, :], STRI, X[:, 1, :],
                             start=True, stop=False)
            nc.tensor.matmul(C1p[:, 1, :], ONESM, X[:, 0, :],
                             start=False, stop=True)
            # r_sum aligned: RSA[t] = RSN[t-1], RSA[0] = S0 const
            RSAp = psmisc.tile([128, NCHUNK, BLOC], F32, tag="misc")
            nc.tensor.matmul(RSAp[:, 0, :], SHM, RSN[:, 0, :],
                             start=True, stop=False)
            nc.tensor.matmul(RSAp[:, 0, :], ONE0[:], CST0[:],
                             start=False, stop=True)
            nc.tensor.matmul(RSAp[:, 1, :], SHM, RSN[:, 1, :],
                             start=True, stop=False)
            nc.tensor.matmul(RSAp[:, 1, :], SH2, RSN[:, 0, :],
                             start=False, stop=True)
            # n_other = c1 + x*(t - 2*c1); notmask = n_other < HALF
            NM = spool.tile([128, NCHUNK, BLOC], F32, tag="NM")
            UT = spool.tile([128, NCHUNK, BLOC], F32, tag="UT")
            for c in range(NCHUNK):
                nc.vector.tensor_scalar(UT[:, c, :], C1p[:, c, :], -2.0,
                                        TV[:, c:c + 1], ALU.mult, ALU.add)
                nc.vector.tensor_tensor(UT[:, c, :], UT[:, c, :], X[:, c, :],
                                        ALU.mult)
                nc.vector.tensor_tensor(UT[:, c, :], UT[:, c, :], C1p[:, c, :],
                                        ALU.add)
                nc.vector.tensor_single_scalar(NM[:, c, :], UT[:, c, :],
                                               float(HALF) - 0.5, ALU.is_lt)
            # term = notmask * (rp - mx - 0.5*softplus(2*(mn-mx)))
            RO = spool.tile([128, NCHUNK, BLOC], F32, tag="RO")
            MX = spool.tile([128, NCHUNK, BLOC], F32, tag="MX")
            MN = spool.tile([128, NCHUNK, BLOC], F32, tag="MN")
            SPt = spool.tile([128, NCHUNK, BLOC], F32, tag="SPt")
            TERM = spool.tile([128, NCHUNK, BLOC], F32, tag="TERM")
            nc.vector.tensor_tensor(RO[:], RSAp[:], RP[:], ALU.subtract)
            nc.vector.tensor_tensor(MX[:], RP[:], RO[:], ALU.max)
            nc.vector.tensor_tensor(MN[:], RP[:], RO[:], ALU.min)
            nc.vector.tensor_tensor(MN[:], MN[:], MX[:], ALU.subtract)
            # softplus(2*(mn-mx)) = ln(1 + exp(2*(mn-mx))) via Exp then Ln(x+1)
            nc.scalar.activation(SPt[:], MN[:], AF.Exp, scale=2.0)
            nc.scalar.activation(SPt[:], SPt[:], AF.Ln, bias=1.0)
            nc.vector.tensor_tensor(MX[:], RP[:], MX[:], ALU.subtract)
            nc.vector.scalar_tensor_tensor(TERM[:], SPt[:], -0.5, MX[:],
                                           ALU.mult, ALU.add)
            nc.vector.tensor_tensor(TERM[:], TERM[:], NM[:], ALU.mult)
            # out[b] = sum_t term
            YPp = psmisc.tile([1, NCHUNK * BLOC], F32, tag="misc")
            nc.tensor.matmul(YPp[:], ONESM[:, 0:1],
                             TERM[:].rearrange("p a b -> p (a b)"),
                             start=True, stop=True)
            YS = spool.tile([1, NCHUNK * BLOC], F32, tag="YS")
            nc.scalar.activation(YS[:], YPp[:], AF.Copy)
            YF = spool.tile([1, BLOC], F32, tag="YF")
            nc.vector.tensor_tensor(YF[:], YS[0:1, 0:BLOC],
                                    YS[0:1, BLOC:2 * BLOC], ALU.add)
            nc.sync.dma_start(y[:], YF[:])
    nc.compile()
    return nc


def host_tables(inputs, epsilon):
    x = np.asarray(inputs).astype(np.float32)        # (B, L)
    eps = np.asarray(epsilon).astype(np.float32)     # (2, M, L)
    eps0, eps1 = eps[0], eps[1]
    le0 = np.log(eps0)                               # (M, L)
    le1 = np.log(eps1)
    w = eps0 + eps1
    lnw_sh = np.zeros((M, L), np.float32)   # now the *linear* shifted weight table
    lnw_sh[:, : L - 1] = w[:, 1:]
    s0 = np.float32(w[:, 0].sum(dtype=np.float64))

    ar = np.arange(128)
    tri = np.asarray(ar[:, None] <= ar[None, :], np.float32)
    stri = np.asarray(ar[:, None] < ar[None, :], np.float32)
    onesm = np.ones((128, 128), np.float32)
    shm = np.asarray(ar[:, None] == (ar[None, :] - 1), np.float32)
    sh2 = np.asarray((ar[:, None] == 127) & (ar[None, :] == 0), np.float32)
    tv = (ar[:, None] + 128.0 * np.arange(NCHUNK)[None, :]).astype(np.float32)

    def chunked(a_t):  # (L, K) -> (128, 2*K) with [:, c*K:(c+1)*K] = chunk c
        return np.concatenate([a_t[c * 128:(c + 1) * 128] for c in range(NCHUNK)],
                              axis=1)

    lr_t = np.ascontiguousarray((le1 - le0).T)       # (L, M)
    l0_t = np.ascontiguousarray(le0.T)
    lnw_t = np.ascontiguousarray(lnw_sh.T)
    xt_all = np.ascontiguousarray(x.T)               # (L, B)

    meg_fixed = [chunked(lr_t), chunked(lnw_t), onesm, stri, shm, sh2, tv]
    megr = np.ascontiguousarray(
        np.concatenate([chunked(l0_t), tri, onesm], axis=1))
    tables = {
        "megr": megr,
        "one0": np.asarray(np.arange(BLOC)[None, :] == 0, np.float32),
        "cst0": np.full((1, BLOC), s0, np.float32),
    }
    return tables, meg_fixed, xt_all, chunked


_NC_CACHE = {}


def get_nc():
    if "nc" not in _NC_CACHE:
        _NC_CACHE["nc"] = build_nc()
    return _NC_CACHE["nc"]


def kernel(inputs, epsilon):
    from concourse.bass_utils import run_bass_kernel_spmd

    tables, meg_fixed, xt_all, chunked = host_tables(inputs, epsilon)
    nc = get_nc()
    in_maps = []
    for k in range(NCORES):
        m = dict(tables)
        xt_core = np.ascontiguousarray(xt_all[:, k * BLOC:(k + 1) * BLOC])
        m["meg"] = np.ascontiguousarray(
            np.concatenate([chunked(xt_core)] + meg_fixed, axis=1))
        in_maps.append(m)
    res = run_bass_kernel_spmd(nc, in_maps, core_ids=list(range(NCORES)))
    out = np.empty((B,), np.float32)
    for k in range(NCORES):
        out[k * BLOC:(k + 1) * BLOC] = np.asarray(res.results[k]["y"]).reshape(-1)
    return out



# revision 2
# speedup vs baseline: 1.3557x; 1.3557x over previous
"""
Trainium2 Bass kernel v2 for nn_ARqGPS (autoregressive qGPS log-amplitude).

Math:
  H[b,m,t]  = prod_{j<=t} eps_sel[b,m,j],  eps_sel = eps[x[b,t],m,t]
  rp[b,t]   = sum_m H[b,m,t]
  rs[b,t]   = sum_m (eps0+eps1)[m,t] * H[b,m,t-1]   (H[.,.,-1] = 1)
  ro = rs - rp;  term = notmask * (rp - mx - 0.5*log1p(exp(2*(mn-mx))))
  out[b] = sum_t term[b,t]

Device (per core, 128 batch rows, t on partitions in 2 chunks of 128):
  T1a[t,(b,m)] = x*lr          (DVE f16 2x via x-replicated-pair broadcast)
  S = TRI@T1a + I@C0_bc (+ ONES@T1a_c0 for chunk1)   (PE, f16 rhs, f32 psum)
      C0[t,m] = cumsum_t log eps0  -- host precomputed
  H = exp(S)                   (ACT, 1024-free reads from 2-bank psum tiles)
  WH = H*wsh_bc                (gpsimd)
  rp/rsn = binary-tree m-reduction in f16, final 4->1 in f32  (DVE)
  tail: counts via STRI matmul, shift via partition-offset copies, softplus,
  masked sum via ones-column matmul.
"""
import os
import sys

import numpy as np

for _p in ("/opt/trn_rl_repo", os.path.expanduser("~/.axon_site/_ro/trn_rl_repo")):
    if os.path.isdir(_p) and _p not in sys.path:
        sys.path.insert(0, _p)
        break

import concourse.bass as bass
import concourse.bacc as bacc
import concourse.mybir as mybir
from concourse.tile import TileContext

B, L, M = 1024, 256, 128
NCORES = 8
BLOC = B // NCORES          # 128 batch rows per core
HALF = L // 2
WB = 16                     # batch rows per wide block
NWB = BLOC // WB            # 8 wide blocks
FB = WB * M                 # 2048 free elems per chunk per wblock
NCHUNK = 2

F32 = mybir.dt.float32
F16 = mybir.dt.float16
AF = mybir.ActivationFunctionType
ALU = mybir.AluOpType
AX = mybir.AxisListType


def build_nc():
    nc = bacc.Bacc("TRN2", target_bir_lowering=False)
    xr2 = nc.dram_tensor("xr2", (128, NCHUNK * WB * NWB * 2), F16,
                         kind="ExternalInput")          # (t, c, b, 2)
    lrt = nc.dram_tensor("lrt", (128, NCHUNK * M), F16, kind="ExternalInput")
    c0t = nc.dram_tensor("c0t", (128, NCHUNK * M), F16, kind="ExternalInput")
    wsht = nc.dram_tensor("wsht", (128, NCHUNK * M), F16, kind="ExternalInput")
    tri3 = nc.dram_tensor("tri3", (128, 3 * 128), F16, kind="ExternalInput")
    shms = nc.dram_tensor("shms", (128, 2 * 128), F32, kind="ExternalInput")
    tvc = nc.dram_tensor("tvc", (128, 4), F32, kind="ExternalInput")
    one0 = nc.dram_tensor("one0", (1, BLOC), F32, kind="ExternalInput")
    cst0 = nc.dram_tensor("cst0", (1, BLOC), F32, kind="ExternalInput")
    y = nc.dram_tensor("y", (1, BLOC), F32, kind="ExternalOutput")

    with TileContext(nc) as tc:
        with (
            tc.tile_pool(name="const", bufs=1) as cpool,
            tc.tile_pool(name="t1p", bufs=2) as t1pool,
            tc.tile_pool(name="hp", bufs=3) as hpool,
            tc.tile_pool(name="whp", bufs=3) as whpool,
            tc.tile_pool(name="s1p", bufs=3) as s1pool,
            tc.tile_pool(name="red", bufs=1) as rpool,
            tc.tile_pool(name="small", bufs=1) as spool,
            tc.tile_pool(name="ps", bufs=2, space="PSUM") as pspool,
            tc.tile_pool(name="psm", bufs=1, space="PSUM") as psmisc,
        ):
            # ---- constants ----
            XR2 = cpool.tile([128, NCHUNK * BLOC * 2], F16, tag="XR2")
            LRT = cpool.tile([128, NCHUNK * M], F16, tag="LRT")
            C0T = cpool.tile([128, NCHUNK * M], F16, tag="C0T")
            WSHT = cpool.tile([128, NCHUNK * M], F16, tag="WSHT")
            TRI3 = cpool.tile([128, 3 * 128], F16, tag="TRI3")
            SHMS = cpool.tile([128, 2 * 128], F32, tag="SHMS")
            TVC = cpool.tile([128, 4], F32, tag="TVC")
            ONE0 = cpool.tile([1, BLOC], F32, tag="ONE0")
            CST0 = cpool.tile([1, BLOC], F32, tag="CST0")
            # critical-path tables first; xr2 split in 4 to spread queues
            nc.sync.dma_start(LRT[:], lrt[:])
            for dq in range(4):
                qs = slice(dq * 128, (dq + 1) * 128)
                nc.sync.dma_start(XR2[:, qs], xr2[:, qs])
            nc.sync.dma_start(TRI3[:], tri3[:])
            nc.sync.dma_start(C0T[:], c0t[:])
            nc.sync.dma_start(WSHT[:], wsht[:])
            nc.sync.dma_start(TVC[:], tvc[:])
            nc.sync.dma_start(ONE0[:], one0[:])
            nc.sync.dma_start(CST0[:], cst0[:])
            nc.sync.dma_start(SHMS[:], shms[:])
            TRI = TRI3[:, 0:128]
            STRI = TRI3[:, 128:256]
            IDT = TRI3[:, 256:384]
            SHM = SHMS[:, 0:128]
            SH2 = SHMS[:, 128:256]
            ONES16 = cpool.tile([128, 128], F16, tag="ONES16")
            nc.vector.memset(ONES16[:], 1.0)
            ONEC = cpool.tile([128, 1], F32, tag="ONEC")
            nc.vector.memset(ONEC[:], 1.0)
            # (t, c, b, 2) view of XR2
            XRV = XR2[:].rearrange("p (c b two) -> p c b two", c=NCHUNK, two=2)

            # ---- persistent reduction tensors ----
            R32 = rpool.tile([128, NCHUNK, BLOC, 32], F16, tag="R32")
            W32 = rpool.tile([128, NCHUNK, BLOC, 32], F16, tag="W32")
            RP = spool.tile([128, NCHUNK, BLOC], F32, tag="RP")
            RSN = spool.tile([128, NCHUNK, BLOC], F32, tag="RSN")

            # ---- main loop ----
            for wb in range(NWB):
                bsl = slice(wb * WB, (wb + 1) * WB)
                t1 = []
                for c in range(NCHUNK):
                    t = t1pool.tile([128, WB, 64, 2], F16, tag=f"T1_{c}")
                    xbc = (XRV[:, c, bsl, :].unsqueeze(2)
                           .broadcast_to([128, WB, 64, 2]))
                    lbc = (LRT[:, c * M:(c + 1) * M]
                           .rearrange("p (a two) -> p a two", two=2)
                           .unsqueeze(1).broadcast_to([128, WB, 64, 2]))
                    # on gpsimd: output feeds only the PE, keeping the DVE
                    # from ever reading Pool-written tiles
                    nc.gpsimd.tensor_tensor(t[:], xbc, lbc, ALU.mult)
                    t1.append(t)
                ht = []
                for c in range(NCHUNK):
                    h = hpool.tile([128, WB, M], F16, tag=f"H_{c}")
                    t1f = t1[c][:].rearrange("p b a two -> p (b a two)")
                    t1f0 = t1[0][:].rearrange("p b a two -> p (b a two)")
                    c0bc = (C0T[:, c * M:(c + 1) * M].unsqueeze(1)
                            .broadcast_to([128, 4, M]))
                    P0 = pspool.tile([128, 1024], F32, tag="S")
                    P1 = pspool.tile([128, 1024], F32, tag="S")
                    P = [P0, P1]
                    # 512-wide matmuls (psum bank limit), grouped by weights
                    for q in range(2):
                        for hh in range(2):
                            sl = slice((2 * q + hh) * 512,
                                       (2 * q + hh + 1) * 512)
                            nc.tensor.matmul(P[q][:, hh * 512:(hh + 1) * 512],
                                             TRI, t1f[:, sl],
                                             start=True, stop=False)
                    for q in range(2):
                        for hh in range(2):
                            po = P[q][:, hh * 512:(hh + 1) * 512]
                            nc.tensor.matmul(po, IDT, c0bc,
                                             start=False, stop=(c == 0))
                    if c == 1:
                        for q in range(2):
                            for hh in range(2):
                                sl = slice((2 * q + hh) * 512,
                                           (2 * q + hh + 1) * 512)
                                nc.tensor.matmul(
                                    P[q][:, hh * 512:(hh + 1) * 512],
                                    ONES16, t1f0[:, sl],
                                    start=False, stop=True)
                    for q in range(2):
                        hv = (h[:].rearrange("p b m -> p (b m)")
                              [:, q * 1024:(q + 1) * 1024])
                        nc.scalar.activation(hv, P[q][:], AF.Exp)
                    ht.append(h)
                for c in range(NCHUNK):
                    w = whpool.tile([128, WB, M], F16, tag=f"W_{c}")
                    wbc = (WSHT[:, c * M:(c + 1) * M].unsqueeze(1)
                           .broadcast_to([128, WB, M]))
                    nc.vector.tensor_tensor(w[:], ht[c][:], wbc, ALU.mult)
                    # tree steps 1-2 for rp and rsn
                    s1 = s1pool.tile([128, WB, 64], F16, tag="S1")
                    nc.vector.tensor_tensor(s1[:], ht[c][:, :, 0:64],
                                            ht[c][:, :, 64:128], ALU.add)
                    nc.vector.tensor_tensor(R32[:, c, bsl, :], s1[:, :, 0:32],
                                            s1[:, :, 32:64], ALU.add)
                    w1 = s1pool.tile([128, WB, 64], F16, tag="W1")
                    nc.vector.tensor_tensor(w1[:], w[:, :, 0:64],
                                            w[:, :, 64:128], ALU.add)
                    nc.vector.tensor_tensor(W32[:, c, bsl, :], w1[:, :, 0:32],
                                            w1[:, :, 32:64], ALU.add)

            # ---- global tree steps 3-5 + final reduce ----
            R16 = rpool.tile([128, NCHUNK, BLOC, 16], F16, tag="R16")
            R8 = rpool.tile([128, NCHUNK, BLOC, 8], F16, tag="R8")
            R4 = rpool.tile([128, NCHUNK, BLOC, 4], F16, tag="R4")
            for src, dst in ((R32, RP), (W32, RSN)):
                nc.vector.tensor_tensor(R16[:], src[:, :, :, 0:16],
                                        src[:, :, :, 16:32], ALU.add)
                nc.vector.tensor_tensor(R8[:], R16[:, :, :, 0:8],
                                        R16[:, :, :, 8:16], ALU.add)
                nc.vector.tensor_tensor(R4[:], R8[:, :, :, 0:4],
                                        R8[:, :, :, 4:8], ALU.add)
                nc.vector.tensor_reduce(dst[:], R4[:], AX.X, ALU.add)

            # ---- tail ----
            # exclusive spin-up counts c1[t,b] via strict-lower-tri matmuls
            X0 = XRV[:, 0, :, 0]                  # (128, BLOC) stride-2 view
            X1 = XRV[:, 1, :, 0]
            C1p = psmisc.tile([128, NCHUNK, BLOC], F32, tag="C1p")
            nc.tensor.matmul(C1p[:, 0, :], STRI, X0, start=True, stop=True)
            nc.tensor.matmul(C1p[:, 1, :], STRI, X1, start=True, stop=False)
            nc.tensor.matmul(C1p[:, 1, :], ONES16, X0, start=False, stop=True)
            # r_sum aligned: RSA[t] = RSN[t-1]; RSA[0] = s0 const
            RSA = psmisc.tile([128, NCHUNK, BLOC], F32, tag="RSA")
            nc.tensor.matmul(RSA[:, 0, :], SHM, RSN[:, 0, :],
                             start=True, stop=False)
            nc.tensor.matmul(RSA[:, 0, :], ONE0[:], CST0[:],
                             start=False, stop=True)
            nc.tensor.matmul(RSA[:, 1, :], SHM, RSN[:, 1, :],
                             start=True, stop=False)
            nc.tensor.matmul(RSA[:, 1, :], SH2, RSN[:, 0, :],
                             start=False, stop=True)
            # n_other = c1 + x*(t - 2*c1); notmask = n_other < HALF
            NM = spool.tile([128, NCHUNK, BLOC], F32, tag="NM")
            UT = spool.tile([128, NCHUNK, BLOC], F32, tag="UT")
            for c in range(NCHUNK):
                xc = XRV[:, c, :, 0]
                nc.vector.tensor_scalar(UT[:, c, :], C1p[:, c, :], -2.0,
                                        TVC[:, c:c + 1], ALU.mult, ALU.add)
                nc.vector.tensor_tensor(UT[:, c, :], UT[:, c, :], xc, ALU.mult)
                nc.vector.tensor_tensor(UT[:, c, :], UT[:, c, :], C1p[:, c, :],
                                        ALU.add)
                nc.vector.tensor_single_scalar(NM[:, c, :], UT[:, c, :],
                                               float(HALF) - 0.5, ALU.is_lt)
            # term = notmask * (rp - mx - 0.5*log1p(exp(2*(mn-mx))))
            RO = spool.tile([128, NCHUNK, BLOC], F32, tag="RO")
            MX = spool.tile([128, NCHUNK, BLOC], F32, tag="MX")
            MN = spool.tile([128, NCHUNK, BLOC], F32, tag="MN")
            SPt = spool.tile([128, NCHUNK, BLOC], F32, tag="SPt")
            TERM = spool.tile([128, NCHUNK, BLOC], F32, tag="TERM")
            nc.vector.tensor_tensor(RO[:], RSA[:], RP[:], ALU.subtract)
            nc.vector.tensor_tensor(MX[:], RP[:], RO[:], ALU.max)
            nc.vector.tensor_tensor(MN[:], RP[:], RO[:], ALU.min)
            nc.vector.tensor_tensor(MN[:], MN[:], MX[:], ALU.subtract)
            nc.scalar.activation(SPt[:], MN[:], AF.Exp, scale=2.0)
            nc.scalar.activation(SPt[:], SPt[:], AF.Ln, bias=1.0)
            nc.vector.tensor_tensor(MX[:], RP[:], MX[:], ALU.subtract)
            nc.vector.scalar_tensor_tensor(TERM[:], SPt[:], -0.5, MX[:],
                                           ALU.mult, ALU.add)
            nc.vector.tensor_tensor(TERM[:], TERM[:], NM[:], ALU.mult)
            # out[b] = sum_t term
            YPp = psmisc.tile([1, NCHUNK * BLOC], F32, tag="YPp")
            nc.tensor.matmul(YPp[:], ONEC[:],
                             TERM[:].rearrange("p a b -> p (a b)"),
                             start=True, stop=True)
            YS = spool.tile([1, NCHUNK * BLOC], F32, tag="YS")
            nc.scalar.activation(YS[:], YPp[:], AF.Copy)
            YF = spool.tile([1, BLOC], F32, tag="YF")
            nc.vector.tensor_tensor(YF[:], YS[0:1, 0:BLOC],
                                    YS[0:1, BLOC:2 * BLOC], ALU.add)
            nc.sync.dma_start(y[:], YF[:])
    nc.compile()
    return nc


def host_tables(inputs, epsilon):
    x = np.asarray(inputs).astype(np.float32)        # (B, L)
    eps = np.asarray(epsilon).astype(np.float64)     # (2, M, L)
    eps0, eps1 = eps[0], eps[1]
    le0 = np.log(eps0)                               # (M, L)
    le1 = np.log(eps1)
    lr = (le1 - le0)                                 # (M, L)
    c0 = np.cumsum(le0, axis=1)                      # (M, L)
    w = eps0 + eps1
    wsh = np.zeros((M, L))
    wsh[:, :L - 1] = w[:, 1:]
    s0 = np.float32(w[:, 0].sum())

    def chunked_t(a):   # (M, L) -> (128, 2*M): [:, c*M:(c+1)*M] = a[:, c-chunk].T
        return np.concatenate([a[:, c * 128:(c + 1) * 128].T
                               for c in range(NCHUNK)], axis=1)

    lrt = chunked_t(lr).astype(np.float16)
    c0t = chunked_t(c0).astype(np.float16)
    wsht = chunked_t(wsh).astype(np.float16)

    ar = np.arange(128)
    tri = (ar[:, None] <= ar[None, :]).astype(np.float16)
    stri = (ar[:, None] < ar[None, :]).astype(np.float16)
    idt = (ar[:, None] == ar[None, :]).astype(np.float16)
    tri3 = np.ascontiguousarray(np.concatenate([tri, stri, idt], axis=1))
    shm = (ar[:, None] == (ar[None, :] - 1)).astype(np.float32)
    sh2 = ((ar[:, None] == 127) & (ar[None, :] == 0)).astype(np.float32)
    shms = np.ascontiguousarray(np.concatenate([shm, sh2], axis=1))
    tvc = np.zeros((128, 4), np.float32)
    tvc[:, 0] = ar
    tvc[:, 1] = ar + 128.0

    tables = {
        "lrt": lrt, "c0t": c0t, "wsht": wsht, "tri3": tri3, "shms": shms,
        "tvc": tvc,
        "one0": (np.arange(BLOC)[None, :] == 0).astype(np.float32),
        "cst0": np.full((1, BLOC), s0, np.float32),
    }
    # xr2 per core built later: (128, c, b, 2) f16
    xt = x.T                                         # (L, B)
    return tables, xt


_NC_CACHE = {}


def get_nc():
    if "nc" not in _NC_CACHE:
        _NC_CACHE["nc"] = build_nc()
    return _NC_CACHE["nc"]


def make_xr2(xt, k):
    xc = xt[:, k * BLOC:(k + 1) * BLOC]              # (L, BLOC)
    xr2 = np.empty((128, NCHUNK, BLOC, 2), np.float16)
    for c in range(NCHUNK):
        xr2[:, c, :, 0] = xc[c * 128:(c + 1) * 128]
        xr2[:, c, :, 1] = xc[c * 128:(c + 1) * 128]
    return np.ascontiguousarray(xr2.reshape(128, -1))


def kernel(inputs, epsilon):
    from concourse.bass_utils import run_bass_kernel_spmd

    tables, xt = host_tables(inputs, epsilon)
    nc = get_nc()
    in_maps = []
    for k in range(NCORES):
        m = dict(tables)
        m["xr2"] = make_xr2(xt, k)
        in_maps.append(m)
    res = run_bass_kernel_spmd(nc, in_maps, core_ids=list(range(NCORES)))
    out = np.empty((B,), np.float32)
    for k in range(NCORES):
        out[k * BLOC:(k + 1) * BLOC] = np.asarray(res.results[k]["y"]).reshape(-1)
    return out
